# revision 1
# baseline (speedup 1.0000x reference)
"""Distributed Trainium2 kernel for a single causal attention head.

Problem: x (B=1024, T=256, C=1024) f32; Wq/Wk/Wv (1024, 64) f32.
  q,k,v = x@W*;  wei = softmax(mask(q k^T / sqrt(C)));  out = wei @ v.

Strategy (8 NeuronCores, data-parallel over B, no collectives):
  - Host shards B into 8 x 128 batches and lays each shard out TRANSPOSED:
    xT (C=1024, tok=32768) f32, so the device streams x with the
    contraction dim (C) on SBUF partitions -- no on-device transposes of x.
  - Per 512-token block: qT/kT stacked via one [Wq|Wk]-stationary matmul
    pass, vT via a Wv pass (both f32r = TF32-rate), kT un-stacked to
    partition base 0 with a tiny selector matmul.
  - Attention is computed fully transposed: weiT (tok_k, tok_q) = kT.T@qT,
    exp fused into the PSUM->SBUF copy on the scalar engine, causal mask
    applied multiplicatively, out^T = [1|v].T @ expweiT which also yields
    the softmax denominators in partition row 0 (ones-column trick).
    Denominator reciprocal is broadcast across partitions with a K=1
    ones-stationary matmul.
  - Host gathers out^T shards (64, 32768) and untransposes.
"""

import os
import sys

sys.path.insert(0, os.environ.get("TRN_RL_REPO", "/opt/trn_rl_repo"))

from contextlib import ExitStack

import numpy as np
import ml_dtypes

import concourse.bass as bass
import concourse.tile as tile
from concourse import bacc, mybir

F32 = mybir.dt.float32
F32R = mybir.dt.float32r
BF16 = mybir.dt.bfloat16
AF = mybir.ActivationFunctionType

N_CORES = 8
B, T, C, H = 1024, 256, 1024, 64
B_SH = B // N_CORES          # 128 batches per core
TOK = B_SH * T               # 32768 tokens per core
TB = 512                     # tokens per block (= one PSUM bank of f32)
NBLK = TOK // TB             # 64
NC_TILES = C // 128          # 8 contraction tiles
BPB = TB // T                # 2 batches per block


def _r(ap):
    return ap.bitcast(F32R)


def build_graph():
    nc = bacc.Bacc("TRN2", target_bir_lowering=False, debug=False)

    xT = nc.dram_tensor("xT", [C, TOK], F32, kind="ExternalInput").ap()
    wqk_d = nc.dram_tensor("wqk", [C, 128], BF16, kind="ExternalInput").ap()
    wv_d = nc.dram_tensor("wv", [C, H], BF16, kind="ExternalInput").ap()
    id_d = nc.dram_tensor("ident", [H + 1, H + 1], BF16, kind="ExternalInput").ap()
    mask_d = nc.dram_tensor("mask", [128, 2 * T], BF16, kind="ExternalInput").ap()
    ones_d = nc.dram_tensor("ones", [1, H], F32R, kind="ExternalInput").ap()
    out_d = nc.dram_tensor("out", [H, TOK], F32, kind="ExternalOutput").ap()

    with tile.TileContext(nc) as tc, ExitStack() as ctx:
        const = ctx.enter_context(tc.tile_pool(name="const", bufs=1))
        xpool = ctx.enter_context(tc.tile_pool(name="x", bufs=3))
        ps_qk = ctx.enter_context(tc.tile_pool(name="ps_qk", bufs=2, space="PSUM"))
        ps_v = ctx.enter_context(tc.tile_pool(name="ps_v", bufs=2, space="PSUM"))
        ps_wei = ctx.enter_context(tc.tile_pool(name="ps_wei", bufs=2, space="PSUM"))
        ps_attn = ctx.enter_context(tc.tile_pool(name="ps_attn", bufs=2, space="PSUM"))
        sb_qk = ctx.enter_context(tc.tile_pool(name="sb_qk", bufs=2))
        sb_v = ctx.enter_context(tc.tile_pool(name="sb_v", bufs=2))
        sb_k = ctx.enter_context(tc.tile_pool(name="sb_k", bufs=2))
        sb_v1 = ctx.enter_context(tc.tile_pool(name="sb_v1", bufs=4))
        sb_ew = ctx.enter_context(tc.tile_pool(name="sb_ew", bufs=2))
        sb_r = ctx.enter_context(tc.tile_pool(name="sb_r", bufs=2))
        sb_rb = ctx.enter_context(tc.tile_pool(name="sb_rb", bufs=2))
        sb_o = ctx.enter_context(tc.tile_pool(name="sb_o", bufs=2))

        # ---- constants ----
        wqk_t = const.tile([128, C], BF16)          # c-tiles along free dim
        for c in range(NC_TILES):
            nc.sync.dma_start(wqk_t[:, c * 128:(c + 1) * 128],
                              wqk_d[c * 128:(c + 1) * 128, :])
        wv_t = const.tile([128, NC_TILES * H], BF16)
        for c in range(NC_TILES):
            nc.sync.dma_start(wv_t[:, c * H:(c + 1) * H],
                              wv_d[c * 128:(c + 1) * 128, :])
        id_t = const.tile([H + 1, H + 1], BF16)
        nc.sync.dma_start(id_t[:], id_d[:])
        mask_t = const.tile([128, 2 * T], BF16)
        nc.sync.dma_start(mask_t[:], mask_d[:])
        ones_t = const.tile([1, H], F32R)
        nc.sync.dma_start(ones_t[:], ones_d[:])

        # (c*128+p, t) -> (p, c, t) so one DMA fetches a whole block
        xT3 = xT.rearrange("(a p) t -> p a t", p=128)

        for blk in range(NBLK):
            t0 = blk * TB
            # ---- load a whole block of xT in one SWDGE DMA (casts to bf16)
            xt = xpool.tile([128, NC_TILES, TB], BF16, tag="xt")
            nc.gpsimd.dma_start(xt[:], xT3[:, :, t0:t0 + TB])

            # ---- QKV projections (W stationary, xT streaming) ----
            qk_ps = ps_qk.tile([128, TB], F32)      # rows 0:64 qT, 64:128 kT
            for c in range(NC_TILES):
                nc.tensor.matmul(qk_ps[:], wqk_t[:, c * 128:(c + 1) * 128],
                                 xt[:, c, :], start=(c == 0),
                                 stop=(c == NC_TILES - 1))
            v_ps = ps_v.tile([H, TB], F32)
            for c in range(NC_TILES):
                nc.tensor.matmul(v_ps[:], wv_t[:, c * H:(c + 1) * H],
                                 xt[:, c, :], start=(c == 0),
                                 stop=(c == NC_TILES - 1))
            qk_s = sb_qk.tile([H, TB], BF16)
            nc.vector.tensor_copy(qk_s[:], qk_ps[0:H, :])
            # vT plus a row of ones (so the v-transpose emits [v|1] directly)
            vT_s = sb_v.tile([H + 1, TB], BF16)
            nc.vector.tensor_copy(vT_s[0:H, :], v_ps[:])
            nc.gpsimd.memset(vT_s[H:H + 1, :], 1.0)
            # un-stack kT to partition base 0 (DVE cross-quadrant copy)
            kT_s = sb_k.tile([H, TB], BF16)
            nc.vector.tensor_copy(kT_s[:], qk_ps[H:128, :])
            of_s = sb_o.tile([H, TB], F32)

            # ---- attention, one batch (256 tokens) at a time ----
            for b in range(BPB):
                qb = b * T
                # v natural with ones column = transpose([vT; 1])
                v1s = []
                for j in range(2):
                    vt_ps = ps_attn.tile([128, H + 1], BF16, tag="attn")
                    nc.tensor.transpose(
                        vt_ps[:], vT_s[:, qb + j * 128: qb + (j + 1) * 128],
                        id_t[:])
                    v1 = sb_v1.tile([128, H + 1], BF16)
                    nc.vector.tensor_copy(v1[:], vt_ps[:])
                    v1s.append(v1)
                # weiT[k, q] for k-block j: cols [j*T, (j+1)*T)
                wei_ps = ps_wei.tile([128, 2 * T], F32)
                for j in range(2):
                    nc.tensor.matmul(
                        wei_ps[:, j * T:(j + 1) * T],
                        kT_s[:, qb + j * 128: qb + (j + 1) * 128],
                        qk_s[:, qb:qb + T], start=True, stop=True)
                # exp((q k)/sqrt(C)) fused into PSUM->SBUF, then causal mask
                ew_s = sb_ew.tile([128, 2 * T], BF16)
                nc.scalar.activation(ew_s[:], wei_ps[:], AF.Exp,
                                     scale=1.0 / np.sqrt(np.float32(C)))
                nc.vector.tensor_mul(ew_s[:], ew_s[:], mask_t[:])
                # outT = [1|v].T @ expweiT ; row 0 = softmax denominators
                o_ps = ps_attn.tile([H + 1, T], F32, tag="attn")
                for j in range(2):
                    nc.tensor.matmul(o_ps[:], v1s[j][:],
                                     ew_s[:, j * T:(j + 1) * T],
                                     start=(j == 0), stop=(j == 1))
                recip_s = sb_r.tile([1, T], F32R)
                with nc.allow_low_precision(reason="f32r reciprocal broadcast"):
                    nc.vector.reciprocal(recip_s[:], o_ps[H:H + 1, :])
                rb_ps = ps_attn.tile([H, T], F32, tag="attn")
                nc.tensor.matmul(rb_ps[:], ones_t[:], recip_s[:],
                                 start=True, stop=True)
                rb_s = sb_rb.tile([H, T], F32)
                nc.scalar.copy(rb_s[:], rb_ps[:])
                nc.vector.tensor_mul(of_s[:, qb:qb + T], o_ps[0:H, :], rb_s[:])
            nc.sync.dma_start(out_d[:, t0:t0 + TB], of_s[:])

    nc.compile()
    return nc


_GRAPH = None


def _get_graph():
    global _GRAPH
    if _GRAPH is None:
        _GRAPH = build_graph()
    return _GRAPH


def _make_consts():
    ident = np.eye(H + 1, dtype=np.float32).astype(ml_dtypes.bfloat16)
    mask = np.zeros((128, 2 * T), dtype=np.float32)
    for j in range(2):
        for p in range(128):
            k_tok = j * 128 + p
            mask[p, j * T + k_tok:(j + 1) * T] = 1.0
    mask = mask.astype(ml_dtypes.bfloat16)
    ones = np.ones((1, H), dtype=np.float32)
    return ident, mask, ones


def make_in_maps(x, Wq, Wk, Wv):
    ident, mask, ones = _make_consts()
    wqk = np.concatenate([np.asarray(Wq), np.asarray(Wk)], axis=1)
    wqk = wqk.astype(ml_dtypes.bfloat16)
    wv = np.asarray(Wv).astype(ml_dtypes.bfloat16)

    x = np.asarray(x, dtype=np.float32)
    in_maps = []
    for i in range(N_CORES):
        xsh = x[i * B_SH:(i + 1) * B_SH].reshape(TOK, C)
        xTsh = np.ascontiguousarray(xsh.T)
        in_maps.append({
            "xT": xTsh, "wqk": wqk, "wv": wv,
            "ident": ident, "mask": mask, "ones": ones,
        })
    return in_maps


def _run(x, Wq, Wk, Wv, trace=False):
    from concourse.bass_utils import run_bass_kernel_spmd

    nc = _get_graph()
    in_maps = make_in_maps(x, Wq, Wk, Wv)
    res = run_bass_kernel_spmd(nc, in_maps, list(range(N_CORES)), trace=trace)
    full = np.empty((B, T, H), dtype=np.float32)
    for i in range(N_CORES):
        o = np.asarray(res.results[i]["out"])          # (H, TOK)
        full[i * B_SH:(i + 1) * B_SH] = (
            o.reshape(H, B_SH, T).transpose(1, 2, 0))
    return full, res


def kernel(x, Wq, Wk, Wv):
    full, _ = _run(x, Wq, Wk, Wv, trace=False)
    return full


if __name__ == "__main__":
    build_graph()
    print("graph built + compiled OK")



# revision 2
# speedup vs baseline: 1.1261x; 1.1261x over previous
"""Distributed Trainium2 kernel for a single causal attention head.

Problem: x (B=1024, T=256, C=1024) f32; Wq/Wk/Wv (1024, 64) f32.
  q,k,v = x@W*;  wei = softmax(mask(q k^T / sqrt(C)));  out = wei @ v.

Strategy (8 NeuronCores, data-parallel over B, no collectives):
  - Host shards B into 8 x 128 batches, lays each shard out TRANSPOSED
    and pre-cast to bf16: xT (C=1024, tok=32768), so the device streams
    x with the contraction dim (C) on SBUF partitions via plain HWDGE
    DMA (no on-device transpose / cast) at half the f32 HBM traffic.
  - Per 512-token block: qT/kT stacked via one [Wq|Wk]-stationary
    matmul pass; vT via a column-tiled Wv pass (even c-tiles -> PSUM
    rows 0:64, odd -> 64:128, running concurrently in the PE array),
    summed into vT during the PSUM->SBUF eviction.
  - Attention fully transposed: weiT (tok_k, tok_q) = kT.T@qT computed
    only on the causally-live [k, q] parts (the all-masked quarter is
    skipped), exp fused into the PSUM->SBUF copy on the scalar engine,
    causal mask applied multiplicatively on DVE, outT = [v|1].T @
    expweiT which also yields the softmax denominators in row 64
    (ones-column trick).
  - Normalization (divide by denominators) happens on HOST: the device
    emits the unnormalized outT stacked with the denominator row as a
    bf16 (65, tok) tensor; host divides + untransposes.
"""

import os
import sys

sys.path.insert(0, os.environ.get("TRN_RL_REPO", "/opt/trn_rl_repo"))

from contextlib import ExitStack

import numpy as np
import ml_dtypes

import concourse.bass as bass
import concourse.tile as tile
from concourse import bacc, mybir

F32 = mybir.dt.float32
BF16 = mybir.dt.bfloat16
AF = mybir.ActivationFunctionType

N_CORES = 8
B, T, C, H = 1024, 256, 1024, 64
B_SH = B // N_CORES          # 128 batches per core
TOK = B_SH * T               # 32768 tokens per core
TB = 512                     # tokens per block (= one PSUM bank of f32)
NBLK = TOK // TB             # 64
NC_TILES = C // 128          # 8 contraction tiles
BPB = TB // T                # 2 batches per block
OB = 4                       # blocks per output DMA
EW = 384                     # live wei columns per batch: 256 (k-blk 0) + 128 (k-blk 1)


def build_graph():
    nc = bacc.Bacc("TRN2", target_bir_lowering=False, debug=False)

    # blocked layout: xb[p, blk, c, t] = x[blk*TB + t, c*128 + p], so each
    # partition reads one contiguous 8 KiB run per block (large DMA packets)
    xb = nc.dram_tensor("xb", [128, NBLK, NC_TILES, TB], BF16,
                        kind="ExternalInput").ap()
    wqk_d = nc.dram_tensor("wqk", [C, 128], BF16, kind="ExternalInput").ap()
    wv_d = nc.dram_tensor("wv", [C, H], BF16, kind="ExternalInput").ap()
    id_d = nc.dram_tensor("ident", [H + 1, H + 1], BF16, kind="ExternalInput").ap()
    mask_d = nc.dram_tensor("mask", [128, EW], BF16, kind="ExternalInput").ap()
    out_d = nc.dram_tensor("out", [H + 1, TOK], BF16, kind="ExternalOutput").ap()

    with tile.TileContext(nc) as tc, ExitStack() as ctx:
        const = ctx.enter_context(tc.tile_pool(name="const", bufs=1))
        xpool = ctx.enter_context(tc.tile_pool(name="x", bufs=4))
        ps_qk = ctx.enter_context(tc.tile_pool(name="ps_qk", bufs=2, space="PSUM"))
        ps_v = ctx.enter_context(tc.tile_pool(name="ps_v", bufs=2, space="PSUM"))
        ps_wei = ctx.enter_context(tc.tile_pool(name="ps_wei", bufs=2, space="PSUM"))
        ps_attn = ctx.enter_context(tc.tile_pool(name="ps_attn", bufs=2, space="PSUM"))
        sb_q = ctx.enter_context(tc.tile_pool(name="sb_q", bufs=2))
        sb_k = ctx.enter_context(tc.tile_pool(name="sb_k", bufs=2))
        sb_v = ctx.enter_context(tc.tile_pool(name="sb_v", bufs=2))
        sb_v1 = ctx.enter_context(tc.tile_pool(name="sb_v1", bufs=8))
        sb_vo = ctx.enter_context(tc.tile_pool(name="sb_vo", bufs=2))
        sb_ew = ctx.enter_context(tc.tile_pool(name="sb_ew", bufs=3))
        sb_o = ctx.enter_context(tc.tile_pool(name="sb_o", bufs=2))

        # ---- constants ----
        wqk_t = const.tile([128, C], BF16)          # c-tiles along free dim
        for c in range(NC_TILES):
            nc.sync.dma_start(wqk_t[:, c * 128:(c + 1) * 128],
                              wqk_d[c * 128:(c + 1) * 128, :])
        wv_t = const.tile([128, NC_TILES * H], BF16)
        for c in range(NC_TILES):
            nc.sync.dma_start(wv_t[:, c * H:(c + 1) * H],
                              wv_d[c * 128:(c + 1) * 128, :])
        id_t = const.tile([H + 1, H + 1], BF16)
        nc.sync.dma_start(id_t[:], id_d[:])
        mask_t = const.tile([128, EW], BF16)
        nc.sync.dma_start(mask_t[:], mask_d[:])

        # prefetch the first x blocks right behind the (small) constants
        xts = {}
        for blk in range(2):
            xt_pre = xpool.tile([128, NC_TILES, TB], BF16, tag="xt",
                                name=f"xt_pre{blk}")
            nc.sync.dma_start(xt_pre[:], xb[:, blk, :, :])
            xts[blk] = xt_pre

        for blk in range(NBLK):
            t0 = blk * TB
            if blk % OB == 0:
                of_s = sb_o.tile([H + 1, OB * TB], BF16, tag="of")
            oc0 = (blk % OB) * TB

            # ---- load a whole block of x in one HWDGE DMA (bf16) ----
            if blk in xts:
                xt = xts.pop(blk)
            else:
                xt = xpool.tile([128, NC_TILES, TB], BF16, tag="xt")
                nc.sync.dma_start(xt[:], xb[:, blk, :, :])

            # ---- QKV projections (W stationary, xT streaming) ----
            qk_ps = ps_qk.tile([128, TB], F32)      # rows 0:64 qT, 64:128 kT
            for c in range(NC_TILES):
                nc.tensor.matmul(qk_ps[:], wqk_t[:, c * 128:(c + 1) * 128],
                                 xt[:, c, :], start=(c == 0),
                                 stop=(c == NC_TILES - 1))
            # vT: column-tiled pairs — even c-tiles into rows 0:64, odd
            # c-tiles into rows 64:128; the two column groups run
            # concurrently in the PE array (halves the v pass time).
            v_ps = ps_v.tile([128, TB], F32)
            for c in range(0, NC_TILES, 2):
                nc.tensor.matmul(v_ps[0:H, :], wv_t[:, c * H:(c + 1) * H],
                                 xt[:, c, :], start=(c == 0),
                                 stop=(c == NC_TILES - 2),
                                 tile_position=(0, 0))
            for c in range(1, NC_TILES, 2):
                nc.tensor.matmul(v_ps[H:128, :], wv_t[:, c * H:(c + 1) * H],
                                 xt[:, c, :], start=(c == 1),
                                 stop=(c == NC_TILES - 1),
                                 tile_position=(0, H))
            # evictions: qT + odd-v on ACT, kT on DVE (cross-partition),
            # vT = even half (PSUM) + odd half (SBUF) summed on DVE
            q_s = sb_q.tile([H, TB], BF16)
            nc.scalar.copy(q_s[:], qk_ps[0:H, :])
            kT_s = sb_k.tile([H, TB], BF16)
            nc.vector.tensor_copy(kT_s[:], qk_ps[H:128, :])
            vo_s = sb_vo.tile([H, TB], F32)
            nc.scalar.copy(vo_s[:], v_ps[H:128, :])
            vT1_s = sb_v.tile([H + 1, TB], BF16)
            nc.vector.tensor_add(vT1_s[0:H, :], v_ps[0:H, :], vo_s[:])
            nc.gpsimd.memset(vT1_s[H:H + 1, :], 1.0)

            # v natural with ones column = transpose([vT; 1]) per 128 toks
            v1s = []
            for g in range(2 * BPB):
                vt_ps = ps_attn.tile([128, H + 1], BF16, tag="attn")
                nc.tensor.transpose(
                    vt_ps[:], vT1_s[:, g * 128:(g + 1) * 128], id_t[:])
                v1 = sb_v1.tile([128, H + 1], BF16)
                nc.vector.tensor_copy(v1[:], vt_ps[:])
                v1s.append(v1)

            # ---- attention, one batch (256 tokens) at a time ----
            for b in range(BPB):
                qb = b * T
                # weiT[k, q]: k-blk 0 over all 256 q; k-blk 1 only over
                # q >= 128 (cols 0:127 would be fully masked)
                wei_ps = ps_wei.tile([128, EW], F32)
                nc.tensor.matmul(wei_ps[:, 0:T],
                                 kT_s[:, qb:qb + 128],
                                 q_s[:, qb:qb + T], start=True, stop=True)
                nc.tensor.matmul(wei_ps[:, T:EW],
                                 kT_s[:, qb + 128:qb + T],
                                 q_s[:, qb + 128:qb + T], start=True, stop=True)
                # exp((q k)/sqrt(C)) fused into PSUM->SBUF, then causal mask
                ew_s = sb_ew.tile([128, EW], BF16)
                nc.scalar.activation(ew_s[:], wei_ps[:], AF.Exp,
                                     scale=1.0 / np.sqrt(np.float32(C)))
                nc.vector.tensor_mul(ew_s[:], ew_s[:], mask_t[:])
                # outT = [v|1].T @ expweiT ; row 64 = softmax denominators
                o_ps = ps_attn.tile([H + 1, T], F32, tag="attn")
                nc.tensor.matmul(o_ps[:, 0:128], v1s[2 * b][:],
                                 ew_s[:, 0:128], start=True, stop=True)
                nc.tensor.matmul(o_ps[:, 128:T], v1s[2 * b][:],
                                 ew_s[:, 128:T], start=True, stop=False)
                nc.tensor.matmul(o_ps[:, 128:T], v1s[2 * b + 1][:],
                                 ew_s[:, T:EW], start=False, stop=True)
                nc.scalar.copy(of_s[:, oc0 + qb:oc0 + qb + T], o_ps[:])

            if blk % OB == OB - 1:
                # scalar (ACT) HWDGE ring — separate FIFO from the x loads
                nc.scalar.dma_start(
                    out_d[:, t0 + TB - OB * TB:t0 + TB], of_s[:])

    nc.compile()
    return nc


_GRAPH = None


def _get_graph():
    global _GRAPH
    if _GRAPH is None:
        _GRAPH = build_graph()
    return _GRAPH


def _make_consts():
    ident = np.eye(H + 1, dtype=np.float32).astype(ml_dtypes.bfloat16)
    # mask for ew layout [128, 384]: cols 0:256 = k-blk 0 (k_tok = p) over
    # q = 0..255; cols 256:384 = k-blk 1 (k_tok = 128+p) over q = 128..255.
    mask = np.zeros((128, EW), dtype=np.float32)
    for p in range(128):
        mask[p, p:T] = 1.0
        mask[p, T + p:EW] = 1.0
    mask = mask.astype(ml_dtypes.bfloat16)
    return ident, mask


def make_in_maps(x, Wq, Wk, Wv):
    ident, mask = _make_consts()
    wqk = np.concatenate([np.asarray(Wq), np.asarray(Wk)], axis=1)
    wqk = wqk.astype(ml_dtypes.bfloat16)
    wv = np.asarray(Wv).astype(ml_dtypes.bfloat16)

    x = np.asarray(x, dtype=np.float32)
    in_maps = []
    for i in range(N_CORES):
        xsh = x[i * B_SH:(i + 1) * B_SH].reshape(TOK, C)
        # xb[p, blk, c, t] = xsh[blk*TB + t, c*128 + p]
        xbi = np.ascontiguousarray(
            xsh.reshape(NBLK, TB, NC_TILES, 128).transpose(3, 0, 2, 1)
        ).astype(ml_dtypes.bfloat16)
        in_maps.append({
            "xb": xbi, "wqk": wqk, "wv": wv,
            "ident": ident, "mask": mask,
        })
    return in_maps


def _postprocess(results):
    full = np.empty((B, T, H), dtype=np.float32)
    for i in range(N_CORES):
        o = np.asarray(results[i]["out"]).astype(np.float32)   # (65, TOK)
        o = o[0:H] / o[H:H + 1]
        full[i * B_SH:(i + 1) * B_SH] = (
            o.reshape(H, B_SH, T).transpose(1, 2, 0))
    return full


def _run(x, Wq, Wk, Wv, trace=False):
    from concourse.bass_utils import run_bass_kernel_spmd

    nc = _get_graph()
    in_maps = make_in_maps(x, Wq, Wk, Wv)
    res = run_bass_kernel_spmd(nc, in_maps, list(range(N_CORES)), trace=trace)
    return _postprocess(res.results), res


def kernel(x, Wq, Wk, Wv):
    full, _ = _run(x, Wq, Wk, Wv, trace=False)
    return full


if __name__ == "__main__":
    build_graph()
    print("graph built + compiled OK")
